# revision 28
# baseline (speedup 1.0000x reference)
"""Trainium2 Bass kernel for nn_Decoder_51539607552479.

DecoderModule.forward: bilinear-upsample xt (32->64, align_corners) ->
xfuse = xup + alpha*xm -> conv3x3(512->512)+BN+ReLU -> conv3x3(512->256)
+BN+ReLU.  Pure data parallel: batch dim (8) across the 8 NeuronCores,
weights replicated.

1D Winograd F(2,3) along x (direct 3-tap accumulation along y in PSUM)
with bf16 matmuls: 12 accumulating matmuls per output row-tile per
x-position instead of 36 direct taps -> 1.5x fewer PE rows; bf16
elementwise ops run in DVE 2x/4x perf modes (contiguous last dim).

dataflow per core (one image):
  DVE : bilinear upsample+fuse (bf16, parity-blocked cols) -> xloc tiles;
        inverse transform y0=m0+m1+m2, y1=m1-m2-m3 (+BN shift, ReLU)
        from Act-drained bf16 m -> eo tiles / output staging.
  Pool: input transforms V[p] = B^T d (4 col combos per row) from
        xloc/eo tiles into persistent pos-major V planes (no halo:
        x-transform is per-row; y-taps read adjacent plane rows).
  PE  : m_p += U[p,dy]^T V[p] (4 pos x 3 dy x 4 ci accumulating
        matmuls per row tile per co-tile), bf16.
  Act : PSUM->bf16 SBUF drain of all 4 positions in one op.
  host: Winograd weight transform (G w along x), BN scale folded into
        weights, alpha folded into xm, bf16 casts, parity split (free).
"""
import sys

if '/opt/trn_rl_repo' not in sys.path:
    sys.path.insert(0, '/opt/trn_rl_repo')

import numpy as np
import ml_dtypes
import concourse.bacc as bacc
import concourse.mybir as mybir
from concourse.ap import AP
from concourse.tile import TileContext
from concourse.bass_utils import run_bass_kernel_spmd

F32 = mybir.dt.float32
BF16 = mybir.dt.bfloat16
ALU = mybir.AluOpType
ACTF = mybir.ActivationFunctionType
EPS = 1e-5
BF = ml_dtypes.bfloat16

ROW_TILES = [(6 * i, 6) for i in range(10)] + [(60, 4)]
N_CORES = 8
UP_CHUNKS = [(0, 16), (16, 16), (32, 16), (48, 16)]
VP = 66 * 32               # one position's rows in a V plane
# xloc/eo blocked layout: cols 0..32 = even-x plane E[t] (x=2t, pad t=32),
# cols 33..65 = odd-x plane O[t] (x=2t-1, pad t=0)
EB = 0                     # E base col
OB = 33                    # O base col


def _v(ap2d, offset, rows, rowstep, cols):
    """[128, rows, cols] strided view of a [128, L] AP starting at offset."""
    sl = ap2d[:, offset: offset + rows * rowstep]
    return sl.rearrange("p (r c) -> p r c", c=rowstep)[:, :, 0:cols]


def build_nc():
    nc = bacc.Bacc(None, target_bir_lowering=True)

    xt_d = nc.dram_tensor("xt", [512, 1024], BF16, kind="ExternalInput")
    xm_d = nc.dram_tensor("xmeo", [512, 4096], BF16, kind="ExternalInput")
    patx_d = nc.dram_tensor("patx", [64], BF16, kind="ExternalInput")
    paty_d = nc.dram_tensor("paty", [64 * 64], BF16, kind="ExternalInput")
    w0_d = nc.dram_tensor("w0p", [16 * 128, 12 * 128], BF16,
                          kind="ExternalInput")
    w1_d = nc.dram_tensor("w1p", [8 * 128, 12 * 128], BF16,
                          kind="ExternalInput")
    bn_d = {}
    for nm in ("g0", "b0", "m0", "v0"):
        bn_d[nm] = nc.dram_tensor(nm, [512, 1], F32, kind="ExternalInput")
    for nm in ("g1", "b1", "m1", "v1"):
        bn_d[nm] = nc.dram_tensor(nm, [256, 1], F32, kind="ExternalInput")
    out_d = nc.dram_tensor("out", [256, 4096], F32, kind="ExternalOutput")

    with TileContext(nc) as tc:
        with tc.tile_pool(name="main", bufs=1) as P, \
             tc.tile_pool(name="wp", bufs=9) as WP, \
             tc.tile_pool(name="xtp", bufs=2) as XTP, \
             tc.tile_pool(name="xmp", bufs=2) as XMP, \
             tc.tile_pool(name="xhp", bufs=1) as XHP, \
             tc.tile_pool(name="xlp", bufs=2) as XLP, \
             tc.tile_pool(name="eop", bufs=4) as EOP, \
             tc.tile_pool(name="tmp", bufs=1) as TMP, \
             tc.tile_pool(name="mbp", bufs=6) as MBP, \
             tc.tile_pool(name="sbp", bufs=2) as SBP, \
             tc.tile_pool(name="outp", bufs=2) as OUTP, \
             tc.tile_pool(name="psum", bufs=4, space="PSUM") as PS:

            # ---------------- DRAM fetch helpers ----------------
            xt_tiles = {}
            xm_tiles = {}

            def xt_rows(c):
                r0, nr = UP_CHUNKS[c]
                j0, j1 = r0 // 2, (r0 + nr) // 2
                return max(j0 - 1, 0), min(j1 + 1, 32)

            def fetch_chunk(ct, c):
                jstart, jstop = xt_rows(c)
                ny = jstop - jstart
                t = XTP.tile([128, 10 * 32], BF16, tag="xt",
                             name=f"xt{ct}_{c}")
                nc.sync.dma_start(t[:, 0:ny * 32],
                                  xt_d[ct * 128:(ct + 1) * 128,
                                       jstart * 32: jstop * 32])
                xt_tiles[(ct, c)] = t
                r0, nr = UP_CHUNKS[c]
                tm = XMP.tile([128, 16 * 64 + 160], BF16, tag="xm",
                              name=f"xm{ct}_{c}")
                nc.sync.dma_start(tm[:, 0:nr * 64],
                                  xm_d[ct * 128:(ct + 1) * 128,
                                       r0 * 64:(r0 + nr) * 64])
                xm_tiles[(ct, c)] = tm

            patx = P.tile([128, 64], BF16, tag="patx")
            nc.sync.dma_start(patx[:], patx_d[:].partition_broadcast(128))
            paty = P.tile([128, 64 * 64], BF16, tag="paty")
            nc.sync.dma_start(paty[:], paty_d[:].partition_broadcast(128))

            def _cap(tile, off, dims):
                base = tile[:]
                pdim = list(list(base.ap)[0])
                return AP(base.tensor, base.offset + off,
                          [pdim] + [[s, n] for s, n in dims])
            for ct in range(4):
                fetch_chunk(ct, 0)

            # conv0 sweep-A weights (co-tiles 0,1)
            wt = {}

            def fetch_w(layer, qs):
                w_dram = w0_d if layer == 0 else w1_d
                for q in qs:
                    for ci in range(4):
                        t = WP.tile([128, 12 * 128], BF16, tag="w",
                                    name=f"w{layer}_{q}_{ci}")
                        row0 = (q * 4 + ci) * 128
                        nc.sync.dma_start(t[:], w_dram[row0:row0 + 128, :])
                        wt[(layer, q, ci)] = t

            fetch_w(0, (0, 1))

            bnp = {}
            for layer, n_cot in ((0, 4), (1, 2)):
                for q in range(n_cot):
                    sl = slice(q * 128, (q + 1) * 128)
                    for nm in ("g", "b", "m", "v"):
                        t = P.tile([128, 1], F32, tag=f"bn{nm}{layer}_{q}")
                        nc.sync.dma_start(t[:], bn_d[f"{nm}{layer}"][sl, :])
                        bnp[(layer, q, nm)] = t

            # ---------------- PE warmup ----------------
            wscr = P.tile([128, 640], BF16, tag="wscr")
            nc.gpsimd.memset(wscr[:], 0.0)
            pw = PS.tile([128, 1024], F32, tag="cpsum", name="pwarm")
            for wi in range(46):
                nc.tensor.matmul(pw[:, 0:512], wscr[:, 0:128],
                                 wscr[:, 128:640],
                                 start=True, stop=True, skip_group_check=True)

            # ---------------- persistent V planes ----------------
            # V plane: [128, 4 pos * 66 rows * 32], row pr = image row + 1
            V0 = [P.tile([128, 4 * VP], BF16, tag=f"v0_{i}", name=f"v0_{i}")
                  for i in range(4)]
            V1 = [P.tile([128, 4 * VP], BF16, tag=f"v1_{i}", name=f"v1_{i}")
                  for i in range(4)]
            for t in V0 + V1:
                # zero pad rows (pr 0 and 65) for all 4 positions.
                # True memset (not mul-by-0: SBUF garbage can be NaN).
                tv = t[:, 0:4 * VP].rearrange("p (g r) -> p g r", r=VP)
                nc.gpsimd.memset(tv[:, :, 0:32], 0.0)
                nc.gpsimd.memset(tv[:, :, 65 * 32:VP], 0.0)

            # ---------------- BN shift vectors ----------------
            shift = {}

            def emit_bn():
                for layer, n_cot in ((0, 4), (1, 2)):
                    for q in range(n_cot):
                        t = P.tile([128, 1], F32, tag=f"bnt{layer}_{q}")
                        sh = P.tile([128, 1], F32, tag=f"sh{layer}_{q}")
                        nc.vector.tensor_scalar_add(
                            t[:], bnp[(layer, q, "v")][:], EPS)
                        nc.scalar.activation(t[:], t[:], ACTF.Sqrt)
                        nc.vector.reciprocal(t[:], t[:])
                        nc.vector.tensor_mul(t[:], bnp[(layer, q, "g")][:],
                                             t[:])
                        nc.vector.tensor_mul(t[:], bnp[(layer, q, "m")][:],
                                             t[:])
                        nc.vector.tensor_sub(sh[:], bnp[(layer, q, "b")][:],
                                             t[:])
                        shift[(layer, q)] = sh

            # ---------------- x transform (B^T d), Pool ----------------
            def v_rows(src, splane, prow0, nrows, rstep, name,
                       eng=None):
                """src: blocked E|O tile view base AP [128, L]; writes
                V[p] rows prow0..prow0+nrows of plane splane."""
                eng = eng or nc.gpsimd
                E = _v(src, EB, nrows, rstep, 33)
                O = _v(src, OB, nrows, rstep, 33)
                vv = [_v(splane, p * VP + prow0 * 32, nrows, 32, 32)
                      for p in range(4)]
                eng.tensor_sub(vv[0], O[:, :, 0:32], O[:, :, 1:33])
                eng.tensor_add(vv[1], E[:, :, 0:32], O[:, :, 1:33])
                eng.tensor_sub(vv[2], O[:, :, 1:33], E[:, :, 0:32])
                eng.tensor_sub(vv[3], E[:, :, 0:32], E[:, :, 1:33])

            # ---------------- upsample + fuse (DVE, bf16) ----------------
            def upsample_chunk(ct, c, eng=None):
                if (ct, c) not in xt_tiles:
                    fetch_chunk(ct, c)
                r0, nrow = UP_CHUNKS[c]
                j0, j1 = r0 // 2, (r0 + nrow) // 2
                jstart, jstop = xt_rows(c)
                ny = jstop - jstart
                nD = ny - 1
                xt4 = xt_tiles[(ct, c)][:, 0:ny * 32] \
                    .rearrange("p (r c) -> p r c", c=32)
                xm_t = xm_tiles[(ct, c)]
                eng = eng or nc.vector
                # ---- x interp -> xh blocked [p, ny, 64] (E 0:32, O 32:64)
                # d layout [p, ny, 33]: col0 pad, cols1..32 = d[0..31]
                d = TMP.tile([128, 10 * 33], BF16, tag="d")
                d3 = d[:, 0:ny * 33].rearrange("p (r c) -> p r c", c=33)
                nc.gpsimd.memset(_cap(d, 0, [(33, ny), (32, 2)]), 0.0)
                eng.tensor_sub(d3[:, :, 1:32], xt4[:, :, 1:32],
                               xt4[:, :, 0:31])
                xh = XHP.tile([128, 10 * 64], BF16, tag="xh")
                xh4 = xh[:, 0:ny * 64].rearrange("p (r c) -> p r c", c=64)
                # merged E|O: xh[., s, k] = xt[., k] + wx[s, k]*d3[., s+k]
                xh_eo = _cap(xh, 0, [(64, ny), (32, 2), (1, 32)])
                eng.tensor_mul(xh_eo,
                               _cap(d, 0, [(33, ny), (1, 2), (1, 32)]),
                               _cap(patx, 0, [(0, ny), (32, 2), (1, 32)]))
                eng.tensor_add(xh_eo, xh_eo,
                               _cap(xt_tiles[(ct, c)],
                                    (jstart - jstart) * 32,
                                    [(32, ny), (0, 2), (1, 32)]))
                # ---- y interp + fuse -> xloc blocked [p, 16, 66]
                xl = XLP.tile([128, 16 * 66 + 176], BF16, tag="xl",
                              name=f"xl{ct}_{c}")
                nc.gpsimd.memset(_cap(xl, 32, [(66, 16), (1, 2)]), 0.0)
                # Dh rows at slot (j - (j0-1)); slot 0 / last may be dummy
                Dh = TMP.tile([128, 10 * 64], BF16, tag="dh")
                dh_off = 1 if c == 0 else 0
                if c == 0:
                    nc.gpsimd.memset(Dh[:, 0:64], 0.0)     # Dh[-1] (wye[0]=0)
                if c == 3:
                    nc.gpsimd.memset(Dh[:, 8 * 64:9 * 64], 0.0)  # Dh[31]
                eng.tensor_sub(Dh[:, dh_off * 64: (dh_off + nD) * 64],
                               xh[:, 64:(nD + 1) * 64], xh[:, 0:nD * 64])
                # unified even+odd rows: X[2j+k] = xh[j] + wyi[j,k]*Dh[j-1+k]
                #                                + xm[2j+k]
                ty = TMP.tile([128, 16 * 64], BF16, tag="ty")
                ty4 = _cap(ty, 0, [(128, 8), (64, 2), (1, 64)])
                eng.tensor_mul(ty4,
                               _cap(Dh, 0, [(64, 8), (64, 2), (1, 64)]),
                               _cap(paty, j0 * 128,
                                    [(128, 8), (64, 2), (1, 64)]))
                eng.tensor_add(ty4, ty4,
                               _cap(xm_t, 0, [(128, 8), (64, 2), (1, 64)]))
                xo = (j0 - jstart) * 64
                eng.tensor_add(_cap(xl, EB, [(132, 8), (66, 2), (1, 32)]),
                               _cap(xh, xo, [(64, 8), (0, 2), (1, 32)]),
                               _cap(ty, 0, [(128, 8), (64, 2), (1, 32)]))
                eng.tensor_add(_cap(xl, OB + 1, [(132, 8), (66, 2), (1, 32)]),
                               _cap(xh, xo + 32, [(64, 8), (0, 2), (1, 32)]),
                               _cap(ty, 32, [(128, 8), (64, 2), (1, 32)]))
                # ---- V0 rows for this chunk (DVE)
                v_rows(xl, V0[ct], r0 + 1, 16, 66, f"v0_{ct}_{c}",
                       eng=nc.vector)

            # ---------------- conv row-tile ----------------
            def conv_rt(layer, rt, q):
                r0, nr = ROW_TILES[rt]
                vsrc = V0 if layer == 0 else V1
                pt = PS.tile([128, 1024], F32, tag="cpsum",
                             name=f"ps{layer}_{rt}_{q}")
                for p in range(4):
                    ov = pt[:, p * 256: p * 256 + nr * 32] \
                        .rearrange("p (r c) -> p r c", c=32)
                    for dy in range(3):
                        for ci in range(4):
                            lhsT = wt[(layer, q, ci)][:, (p * 3 + dy) * 128:
                                                      (p * 3 + dy + 1) * 128]
                            rhs = _v(vsrc[ci], p * VP + (r0 + dy) * 32,
                                     nr, 32, 32)
                            nc.tensor.matmul(ov, lhsT, rhs,
                                             start=(dy == 0 and ci == 0),
                                             stop=(dy == 2 and ci == 3),
                                             skip_group_check=True)
                # drain 4 positions psum -> bf16 in one Act op
                mb = MBP.tile([128, 4 * 192], BF16, tag="mb",
                              name=f"mb{layer}_{rt}_{q}")
                n = nr * 32
                nc.scalar.activation(
                    mb[:, 0:4 * n].rearrange("p (g c) -> p g c", c=n),
                    _v(pt, 0, 4, 256, n), ACTF.Copy)
                # inverse transform + BN shift + ReLU on DVE
                eng = nc.vector
                # m layout [m0|m1|m2'|m3] with m2' = -m2 (host-negated):
                # [ta1|tb1] = [m0|m1]+[m1|m2'];  [ta|tb] = [ta1|tb1]-[m2'|m3]
                tab = SBP.tile([128, 384], BF16, tag="ta")
                eng.tensor_add(tab[:, 0:2 * n], mb[:, 0:2 * n],
                               mb[:, n:3 * n])
                eng.tensor_sub(tab[:, 0:2 * n], tab[:, 0:2 * n],
                               mb[:, 2 * n:4 * n])
                sh = shift[(layer, q)]
                tav = tab[:, 0:n].rearrange("p (r c) -> p r c", c=32)
                tbv = tab[:, n:2 * n].rearrange("p (r c) -> p r c", c=32)
                if layer == 0:
                    eo = EOP.tile([128, 6 * 66 + 80], BF16, tag="eo",
                                  name=f"eo{rt}_{q}")
                    eng.memset(_v(eo, 32, nr, 66, 2), 0.0)
                    eng.tensor_scalar(_v(eo, EB, nr, 66, 32), tav,
                                      sh[:, 0:1], 0.0, ALU.add, ALU.max)
                    eng.tensor_scalar(_v(eo, OB + 1, nr, 66, 32), tbv,
                                      sh[:, 0:1], 0.0, ALU.add, ALU.max)
                    # conv1 input transform rows for (rt, ci=q) on Pool
                    v_rows(eo, V1[q], r0 + 1, nr, 66, f"v1_{rt}_{q}")
                else:
                    ob = OUTP.tile([128, 384], F32, tag="ob",
                                   name=f"ob_{rt}_{q}")
                    ov4 = ob[:, 0:nr * 64].rearrange("p (r c t) -> p r c t",
                                                     c=32, t=2)
                    eng.tensor_scalar(ov4[:, :, :, 0:1].squeeze(), tav,
                                      sh[:, 0:1], 0.0, ALU.add, ALU.max)
                    eng.tensor_scalar(ov4[:, :, :, 1:2].squeeze(), tbv,
                                      sh[:, 0:1], 0.0, ALU.add, ALU.max)
                    nc.sync.dma_start(
                        out_d[q * 128:(q + 1) * 128,
                              r0 * 64:(r0 + nr) * 64],
                        ob[:, 0:nr * 64])

            # ---------------- emission ----------------
            def up_set(c):
                for ct in range(4):
                    upsample_chunk(ct, c)

            up_set(0)
            up_set(1)
            up_set(2)

            # sweep order: rt-major, but the last two row tiles complete one
            # co-tile at a time so its weight slots free early for the next
            # sweep's DMA.
            def sweep(layer, qa, qb):
                order = [(rt, q) for rt in range(9) for q in (qa, qb)]
                order += [(9, qa), (10, qa), (9, qb), (10, qb)]
                return order

            emit_bn()
            # conv0 sweep A (co-tiles 0,1)
            for rt, q in sweep(0, 0, 1):
                conv_rt(0, rt, q)
                if (rt, q) == (0, 1):
                    up_set(3)
                elif (rt, q) == (2, 1):
                    fetch_w(0, (2, 3))
            # conv0 sweep B (co-tiles 2,3)
            for rt, q in sweep(0, 2, 3):
                conv_rt(0, rt, q)
                if (rt, q) == (3, 3):
                    fetch_w(1, (0, 1))
            # conv1 (co-tiles 0,1 = all 256)
            for rt, q in sweep(1, 0, 1):
                conv_rt(1, rt, q)

    nc.finalize()
    return nc


_CACHED_NC = None


def _get_nc():
    global _CACHED_NC
    if _CACHED_NC is None:
        _CACHED_NC = build_nc()
    return _CACHED_NC


def _pack_wino(w, scale, n_cot):
    """[co, ci, 3, 3] -> [n_cot*4*128, 12*128] bf16: G-transformed along x,
    BN scale folded, blocks (q, ci_t)[ci_in, (p, dy), co_in]."""
    G = np.array([[1, 0, 0], [.5, .5, .5], [.5, -.5, .5], [0, 0, 1]],
                 np.float32)
    U = np.einsum('pk,oidk->pdoi', G, w.astype(np.float32))   # [4,3,co,ci]
    U = U * scale[None, None, :, None]
    U[2] = -U[2]   # stage-B: [ta1|tb1]=[m0|m1]+[m1|m2'], [ta|tb]-=[m2'|m3]
    A = U.transpose(3, 0, 1, 2)                               # [ci,p,dy,co]
    A = A.reshape(4, 128, 4, 3, n_cot, 128)
    A = A.transpose(4, 0, 1, 2, 3, 5)            # q,cit,ciin,p,dy,coin
    return np.ascontiguousarray(A.astype(BF)).reshape(n_cot * 4 * 128,
                                                      12 * 128)


def kernel(**inputs) -> np.ndarray:
    xt = np.asarray(inputs["xt"], np.float32)     # [8,512,32,32]
    xm = np.asarray(inputs["xm"], np.float32)     # [8,512,64,64]
    alpha = float(np.asarray(inputs["alpha"], np.float32).reshape(1)[0])
    w0 = np.asarray(inputs["w0"], np.float32)
    w1 = np.asarray(inputs["w1"], np.float32)
    g0 = np.asarray(inputs["g0"], np.float32)
    v0 = np.asarray(inputs["v0"], np.float32)
    g1 = np.asarray(inputs["g1"], np.float32)
    v1 = np.asarray(inputs["v1"], np.float32)

    s0 = g0 / np.sqrt(v0 + EPS)
    s1 = g1 / np.sqrt(v1 + EPS)
    w0p = _pack_wino(w0, s0, 4)
    w1p = _pack_wino(w1, s1, 2)

    kk = np.arange(0, 32, dtype=np.float32)
    wxeP = np.where(kk >= 1, -(kk / 63.0), 0.0)       # xhE[k]=xt[k]+wxeP[k]*d[k-1]
    wxoP = np.where(kk <= 30, (31 - kk) / 63.0, 0.0)  # xhO[k]=xt[k]+wxoP[k]*d[k]
    patx = np.concatenate([wxeP, wxoP]).astype(BF)
    # wyi: per j, [wye[j] x64 | wyo[j] x64]; wye[0]=0, wyo[31]=0 absorb edges
    wye = np.where(kk >= 1, -(kk / 63.0), 0.0)
    wyo = np.where(kk <= 30, (31 - kk) / 63.0, 0.0)
    paty = np.stack([np.repeat(wye, 64).reshape(32, 64),
                     np.repeat(wyo, 64).reshape(32, 64)],
                    axis=1).reshape(-1).astype(BF)    # [32*2*64]

    # alpha folded into xm; cols parity-blocked [E(32) | O(32)]
    xmeo = np.empty((N_CORES, 512, 64, 64), np.float32)
    xmeo[:, :, :, 0:32] = alpha * xm[:, :, :, 0::2]
    xmeo[:, :, :, 32:64] = alpha * xm[:, :, :, 1::2]
    xmeo = xmeo.astype(BF)
    xtb = xt.astype(BF)

    common = {"patx": patx, "paty": paty, "w0p": w0p, "w1p": w1p}
    for nm in ("g0", "b0", "m0", "v0"):
        common[nm] = np.asarray(inputs[nm], np.float32).reshape(512, 1)
    for nm in ("g1", "b1", "m1", "v1"):
        common[nm] = np.asarray(inputs[nm], np.float32).reshape(256, 1)

    in_maps = []
    for b in range(N_CORES):
        m = dict(common)
        m["xt"] = np.ascontiguousarray(xtb[b].reshape(512, 1024))
        m["xmeo"] = np.ascontiguousarray(xmeo[b].reshape(512, 4096))
        in_maps.append(m)

    nc = _get_nc()
    res = run_bass_kernel_spmd(nc, in_maps, core_ids=list(range(N_CORES)))
    out = np.stack([res.results[b]["out"].reshape(256, 64, 64)
                    for b in range(N_CORES)], axis=0)
    return out.astype(np.float32)


# revision 33
# speedup vs baseline: 1.0039x; 1.0039x over previous
"""Trainium2 Bass kernel for nn_Decoder_51539607552479.

DecoderModule.forward: bilinear-upsample xt (32->64, align_corners) ->
xfuse = xup + alpha*xm -> conv3x3(512->512)+BN+ReLU -> conv3x3(512->256)
+BN+ReLU.  Pure data parallel: batch dim (8) across the 8 NeuronCores,
weights replicated.

1D Winograd F(2,3) along x (direct 3-tap accumulation along y in PSUM)
with bf16 matmuls: 12 accumulating matmuls per output row-tile per
x-position instead of 36 direct taps -> 1.5x fewer PE rows; bf16
elementwise ops run in DVE 2x/4x perf modes (contiguous last dim).

dataflow per core (one image):
  DVE : bilinear upsample+fuse (bf16, parity-blocked cols) -> xloc tiles;
        inverse transform y0=m0+m1+m2, y1=m1-m2-m3 (+BN shift, ReLU)
        from Act-drained bf16 m -> eo tiles / output staging.
  Pool: input transforms V[p] = B^T d (4 col combos per row) from
        xloc/eo tiles into persistent pos-major V planes (no halo:
        x-transform is per-row; y-taps read adjacent plane rows).
  PE  : m_p += U[p,dy]^T V[p] (4 pos x 3 dy x 4 ci accumulating
        matmuls per row tile per co-tile), bf16.
  Act : PSUM->bf16 SBUF drain of all 4 positions in one op.
  host: Winograd weight transform (G w along x), BN scale folded into
        weights, alpha folded into xm, bf16 casts, parity split (free).
"""
import sys

if '/opt/trn_rl_repo' not in sys.path:
    sys.path.insert(0, '/opt/trn_rl_repo')

import numpy as np
import ml_dtypes
import concourse.bacc as bacc
import concourse.mybir as mybir
from concourse.ap import AP
from concourse.tile import TileContext
from concourse.bass_utils import run_bass_kernel_spmd

F32 = mybir.dt.float32
BF16 = mybir.dt.bfloat16
ALU = mybir.AluOpType
ACTF = mybir.ActivationFunctionType
EPS = 1e-5
BF = ml_dtypes.bfloat16

ROW_TILES = [(6 * i, 6) for i in range(10)] + [(60, 4)]
N_CORES = 8
UP_CHUNKS = [(0, 16), (16, 16), (32, 16), (48, 16)]
VP = 66 * 32               # one position's rows in a V plane
# xloc/eo blocked layout: cols 0..32 = even-x plane E[t] (x=2t, pad t=32),
# cols 33..65 = odd-x plane O[t] (x=2t-1, pad t=0)
EB = 0                     # E base col
OB = 33                    # O base col


def _v(ap2d, offset, rows, rowstep, cols):
    """[128, rows, cols] strided view of a [128, L] AP starting at offset."""
    sl = ap2d[:, offset: offset + rows * rowstep]
    return sl.rearrange("p (r c) -> p r c", c=rowstep)[:, :, 0:cols]


def build_nc():
    nc = bacc.Bacc(None, target_bir_lowering=True)

    xt_d = nc.dram_tensor("xt", [512, 1024], BF16, kind="ExternalInput")
    xm_d = nc.dram_tensor("xmeo", [512, 4096], BF16, kind="ExternalInput")
    patx_d = nc.dram_tensor("patx", [64], BF16, kind="ExternalInput")
    paty_d = nc.dram_tensor("paty", [64 * 64], BF16, kind="ExternalInput")
    w0_d = nc.dram_tensor("w0p", [16 * 128, 12 * 128], BF16,
                          kind="ExternalInput")
    w1_d = nc.dram_tensor("w1p", [8 * 128, 12 * 128], BF16,
                          kind="ExternalInput")
    bn_d = {}
    for nm in ("g0", "b0", "m0", "v0"):
        bn_d[nm] = nc.dram_tensor(nm, [512, 1], F32, kind="ExternalInput")
    for nm in ("g1", "b1", "m1", "v1"):
        bn_d[nm] = nc.dram_tensor(nm, [256, 1], F32, kind="ExternalInput")
    out_d = nc.dram_tensor("out", [256, 4096], F32, kind="ExternalOutput")

    with TileContext(nc) as tc:
        with tc.tile_pool(name="main", bufs=1) as P, \
             tc.tile_pool(name="wp", bufs=9) as WP, \
             tc.tile_pool(name="xtp", bufs=2) as XTP, \
             tc.tile_pool(name="xmp", bufs=2) as XMP, \
             tc.tile_pool(name="xhp", bufs=1) as XHP, \
             tc.tile_pool(name="xlp", bufs=2) as XLP, \
             tc.tile_pool(name="eop", bufs=4) as EOP, \
             tc.tile_pool(name="tmp", bufs=1) as TMP, \
             tc.tile_pool(name="mbp", bufs=6) as MBP, \
             tc.tile_pool(name="sbp", bufs=2) as SBP, \
             tc.tile_pool(name="outp", bufs=2) as OUTP, \
             tc.tile_pool(name="psum", bufs=4, space="PSUM") as PS:

            # ---------------- DRAM fetch helpers ----------------
            xt_tiles = {}
            xm_tiles = {}

            def xt_rows(c):
                r0, nr = UP_CHUNKS[c]
                j0, j1 = r0 // 2, (r0 + nr) // 2
                return max(j0 - 1, 0), min(j1 + 1, 32)

            def fetch_chunk(ct, c):
                jstart, jstop = xt_rows(c)
                ny = jstop - jstart
                t = XTP.tile([128, 10 * 32], BF16, tag="xt",
                             name=f"xt{ct}_{c}")
                nc.sync.dma_start(t[:, 0:ny * 32],
                                  xt_d[ct * 128:(ct + 1) * 128,
                                       jstart * 32: jstop * 32])
                xt_tiles[(ct, c)] = t
                r0, nr = UP_CHUNKS[c]
                tm = XMP.tile([128, 16 * 64 + 160], BF16, tag="xm",
                              name=f"xm{ct}_{c}")
                nc.sync.dma_start(tm[:, 0:nr * 64],
                                  xm_d[ct * 128:(ct + 1) * 128,
                                       r0 * 64:(r0 + nr) * 64])
                xm_tiles[(ct, c)] = tm

            patx = P.tile([128, 64], BF16, tag="patx")
            nc.sync.dma_start(patx[:], patx_d[:].partition_broadcast(128))
            paty = P.tile([128, 64 * 64], BF16, tag="paty")
            nc.sync.dma_start(paty[:], paty_d[:].partition_broadcast(128))

            def _cap(tile, off, dims):
                base = tile[:]
                pdim = list(list(base.ap)[0])
                return AP(base.tensor, base.offset + off,
                          [pdim] + [[s, n] for s, n in dims])
            for ct in range(4):
                fetch_chunk(ct, 0)

            # conv0 sweep-A weights (co-tiles 0,1)
            wt = {}

            def fetch_w(layer, qs):
                w_dram = w0_d if layer == 0 else w1_d
                for q in qs:
                    for ci in range(4):
                        t = WP.tile([128, 12 * 128], BF16, tag="w",
                                    name=f"w{layer}_{q}_{ci}")
                        row0 = (q * 4 + ci) * 128
                        nc.sync.dma_start(t[:], w_dram[row0:row0 + 128, :])
                        wt[(layer, q, ci)] = t

            fetch_w(0, (0, 1))

            bnp = {}
            for layer, n_cot in ((0, 4), (1, 2)):
                for q in range(n_cot):
                    sl = slice(q * 128, (q + 1) * 128)
                    for nm in ("g", "b", "m", "v"):
                        t = P.tile([128, 1], F32, tag=f"bn{nm}{layer}_{q}")
                        nc.sync.dma_start(t[:], bn_d[f"{nm}{layer}"][sl, :])
                        bnp[(layer, q, nm)] = t

            # ---------------- PE warmup ----------------
            wscr = P.tile([128, 640], BF16, tag="wscr")
            nc.gpsimd.memset(wscr[:], 0.0)
            pw = PS.tile([128, 1024], F32, tag="cpsum", name="pwarm")
            for wi in range(46):
                nc.tensor.matmul(pw[:, 0:512], wscr[:, 0:128],
                                 wscr[:, 128:640],
                                 start=True, stop=True, skip_group_check=True)

            # ---------------- persistent V planes ----------------
            # V plane: [128, 4 pos * 66 rows * 32], row pr = image row + 1
            V0 = [P.tile([128, 4 * VP], BF16, tag=f"v0_{i}", name=f"v0_{i}")
                  for i in range(4)]
            V1 = [P.tile([128, 4 * VP], BF16, tag=f"v1_{i}", name=f"v1_{i}")
                  for i in range(4)]
            for t in V0 + V1:
                # zero pad rows (pr 0 and 65) for all 4 positions.
                # True memset (not mul-by-0: SBUF garbage can be NaN).
                tv = t[:, 0:4 * VP].rearrange("p (g r) -> p g r", r=VP)
                nc.gpsimd.memset(tv[:, :, 0:32], 0.0)
                nc.gpsimd.memset(tv[:, :, 65 * 32:VP], 0.0)

            # ---------------- BN shift vectors ----------------
            shift = {}

            def emit_bn():
                for layer, n_cot in ((0, 4), (1, 2)):
                    for q in range(n_cot):
                        t = P.tile([128, 1], F32, tag=f"bnt{layer}_{q}")
                        sh = P.tile([128, 1], F32, tag=f"sh{layer}_{q}")
                        nc.vector.tensor_scalar_add(
                            t[:], bnp[(layer, q, "v")][:], EPS)
                        nc.scalar.activation(t[:], t[:], ACTF.Sqrt)
                        nc.vector.reciprocal(t[:], t[:])
                        nc.vector.tensor_mul(t[:], bnp[(layer, q, "g")][:],
                                             t[:])
                        nc.vector.tensor_mul(t[:], bnp[(layer, q, "m")][:],
                                             t[:])
                        nc.vector.tensor_sub(sh[:], bnp[(layer, q, "b")][:],
                                             t[:])
                        shift[(layer, q)] = sh

            # ---------------- x transform (B^T d), Pool ----------------
            def v_rows(src, splane, prow0, nrows, rstep, name,
                       eng=None):
                """src: blocked E|O tile view base AP [128, L]; writes
                V[p] rows prow0..prow0+nrows of plane splane."""
                eng = eng or nc.gpsimd
                E = _v(src, EB, nrows, rstep, 33)
                O = _v(src, OB, nrows, rstep, 33)
                vv = [_v(splane, p * VP + prow0 * 32, nrows, 32, 32)
                      for p in range(4)]
                eng.tensor_sub(vv[0], O[:, :, 0:32], O[:, :, 1:33])
                eng.tensor_add(vv[1], E[:, :, 0:32], O[:, :, 1:33])
                eng.tensor_sub(vv[2], O[:, :, 1:33], E[:, :, 0:32])
                eng.tensor_sub(vv[3], E[:, :, 0:32], E[:, :, 1:33])

            # ---------------- upsample + fuse (DVE, bf16) ----------------
            def upsample_chunk(ct, c, eng=None):
                if (ct, c) not in xt_tiles:
                    fetch_chunk(ct, c)
                r0, nrow = UP_CHUNKS[c]
                j0, j1 = r0 // 2, (r0 + nrow) // 2
                jstart, jstop = xt_rows(c)
                ny = jstop - jstart
                nD = ny - 1
                xt4 = xt_tiles[(ct, c)][:, 0:ny * 32] \
                    .rearrange("p (r c) -> p r c", c=32)
                xm_t = xm_tiles[(ct, c)]
                eng = eng or nc.vector
                # ---- x interp -> xh blocked [p, ny, 64] (E 0:32, O 32:64)
                # d layout [p, ny, 33]: col0 pad, cols1..32 = d[0..31]
                d = TMP.tile([128, 10 * 33], BF16, tag="d")
                d3 = d[:, 0:ny * 33].rearrange("p (r c) -> p r c", c=33)
                nc.gpsimd.memset(_cap(d, 0, [(33, ny), (32, 2)]), 0.0)
                eng.tensor_sub(d3[:, :, 1:32], xt4[:, :, 1:32],
                               xt4[:, :, 0:31])
                xh = XHP.tile([128, 10 * 64], BF16, tag="xh")
                xh4 = xh[:, 0:ny * 64].rearrange("p (r c) -> p r c", c=64)
                # merged E|O: xh[., s, k] = xt[., k] + wx[s, k]*d3[., s+k]
                xh_eo = _cap(xh, 0, [(64, ny), (32, 2), (1, 32)])
                eng.tensor_mul(xh_eo,
                               _cap(d, 0, [(33, ny), (1, 2), (1, 32)]),
                               _cap(patx, 0, [(0, ny), (32, 2), (1, 32)]))
                eng.tensor_add(xh_eo, xh_eo,
                               _cap(xt_tiles[(ct, c)],
                                    (jstart - jstart) * 32,
                                    [(32, ny), (0, 2), (1, 32)]))
                # ---- y interp + fuse -> xloc blocked [p, 16, 66]
                xl = XLP.tile([128, 16 * 66 + 176], BF16, tag="xl",
                              name=f"xl{ct}_{c}")
                nc.gpsimd.memset(_cap(xl, 32, [(66, 16), (1, 2)]), 0.0)
                # Dh rows at slot (j - (j0-1)); slot 0 / last may be dummy
                Dh = TMP.tile([128, 10 * 64], BF16, tag="dh")
                dh_off = 1 if c == 0 else 0
                if c == 0:
                    nc.gpsimd.memset(Dh[:, 0:64], 0.0)     # Dh[-1] (wye[0]=0)
                if c == 3:
                    nc.gpsimd.memset(Dh[:, 8 * 64:9 * 64], 0.0)  # Dh[31]
                eng.tensor_sub(Dh[:, dh_off * 64: (dh_off + nD) * 64],
                               xh[:, 64:(nD + 1) * 64], xh[:, 0:nD * 64])
                # unified even+odd rows: X[2j+k] = xh[j] + wyi[j,k]*Dh[j-1+k]
                #                                + xm[2j+k]
                ty = TMP.tile([128, 16 * 64], BF16, tag="ty")
                ty4 = _cap(ty, 0, [(128, 8), (64, 2), (1, 64)])
                eng.tensor_mul(ty4,
                               _cap(Dh, 0, [(64, 8), (64, 2), (1, 64)]),
                               _cap(paty, j0 * 128,
                                    [(128, 8), (64, 2), (1, 64)]))
                xo = (j0 - jstart) * 64
                eng.tensor_add(ty4, ty4,
                               _cap(xh, xo, [(64, 8), (0, 2), (1, 64)]))
                # dest: E cols at +0, O cols at +34, one 4D op incl +xm
                eng.tensor_add(_cap(xl, EB, [(66, 16), (34, 2), (1, 32)]),
                               _cap(ty, 0, [(64, 16), (32, 2), (1, 32)]),
                               _cap(xm_t, 0, [(64, 16), (32, 2), (1, 32)]))
                # ---- V0 rows for this chunk (DVE)
                v_rows(xl, V0[ct], r0 + 1, 16, 66, f"v0_{ct}_{c}",
                       eng=nc.vector)

            # ---------------- conv row-tile ----------------
            def conv_rt(layer, rt, q):
                r0, nr = ROW_TILES[rt]
                vsrc = V0 if layer == 0 else V1
                pt = PS.tile([128, 1024], F32, tag="cpsum",
                             name=f"ps{layer}_{rt}_{q}")
                for p in range(4):
                    ov = pt[:, p * 256: p * 256 + nr * 32] \
                        .rearrange("p (r c) -> p r c", c=32)
                    for dy in range(3):
                        for ci in range(4):
                            lhsT = wt[(layer, q, ci)][:, (p * 3 + dy) * 128:
                                                      (p * 3 + dy + 1) * 128]
                            rhs = _v(vsrc[ci], p * VP + (r0 + dy) * 32,
                                     nr, 32, 32)
                            nc.tensor.matmul(ov, lhsT, rhs,
                                             start=(dy == 0 and ci == 0),
                                             stop=(dy == 2 and ci == 3),
                                             skip_group_check=True)
                # drain 4 positions psum -> bf16 in one Act op
                mb = MBP.tile([128, 4 * 192], BF16, tag="mb",
                              name=f"mb{layer}_{rt}_{q}")
                n = nr * 32
                nc.scalar.activation(
                    mb[:, 0:4 * n].rearrange("p (g c) -> p g c", c=n),
                    _v(pt, 0, 4, 256, n), ACTF.Copy)
                # inverse transform + BN shift + ReLU on DVE
                eng = nc.vector
                # m layout [m0|m1|m2'|m3] with m2' = -m2 (host-negated):
                # [ta1|tb1] = [m0|m1]+[m1|m2'];  [ta|tb] = [ta1|tb1]-[m2'|m3]
                tab = SBP.tile([128, 384], BF16, tag="ta")
                eng.tensor_add(tab[:, 0:2 * n], mb[:, 0:2 * n],
                               mb[:, n:3 * n])
                eng.tensor_sub(tab[:, 0:2 * n], tab[:, 0:2 * n],
                               mb[:, 2 * n:4 * n])
                sh = shift[(layer, q)]
                tav = tab[:, 0:n].rearrange("p (r c) -> p r c", c=32)
                tbv = tab[:, n:2 * n].rearrange("p (r c) -> p r c", c=32)
                if layer == 0:
                    eo = EOP.tile([128, 6 * 66 + 80], BF16, tag="eo",
                                  name=f"eo{rt}_{q}")
                    eng.memset(_v(eo, 32, nr, 66, 2), 0.0)
                    eng.tensor_scalar(_v(eo, EB, nr, 66, 32), tav,
                                      sh[:, 0:1], 0.0, ALU.add, ALU.max)
                    eng.tensor_scalar(_v(eo, OB + 1, nr, 66, 32), tbv,
                                      sh[:, 0:1], 0.0, ALU.add, ALU.max)
                    # conv1 input transform rows for (rt, ci=q) on Pool
                    v_rows(eo, V1[q], r0 + 1, nr, 66, f"v1_{rt}_{q}")
                else:
                    ob = OUTP.tile([128, 384], F32, tag="ob",
                                   name=f"ob_{rt}_{q}")
                    ov4 = ob[:, 0:nr * 64].rearrange("p (r c t) -> p r c t",
                                                     c=32, t=2)
                    eng.tensor_scalar(ov4[:, :, :, 0:1].squeeze(), tav,
                                      sh[:, 0:1], 0.0, ALU.add, ALU.max)
                    eng.tensor_scalar(ov4[:, :, :, 1:2].squeeze(), tbv,
                                      sh[:, 0:1], 0.0, ALU.add, ALU.max)
                    nc.sync.dma_start(
                        out_d[q * 128:(q + 1) * 128,
                              r0 * 64:(r0 + nr) * 64],
                        ob[:, 0:nr * 64])

            # ---------------- emission ----------------
            def up_set(c):
                for ct in range(4):
                    upsample_chunk(ct, c)

            up_set(0)
            up_set(1)
            up_set(2)

            # sweep order: rt-major, but the last two row tiles complete one
            # co-tile at a time so its weight slots free early for the next
            # sweep's DMA.
            def sweep(layer, qa, qb):
                order = [(rt, q) for rt in range(9) for q in (qa, qb)]
                order += [(9, qa), (10, qa), (9, qb), (10, qb)]
                return order

            emit_bn()
            # conv0 sweep A (co-tiles 0,1)
            for rt, q in sweep(0, 0, 1):
                conv_rt(0, rt, q)
                if (rt, q) == (0, 1):
                    up_set(3)
                elif (rt, q) == (2, 1):
                    fetch_w(0, (2, 3))
            # conv0 sweep B (co-tiles 2,3)
            for rt, q in sweep(0, 2, 3):
                conv_rt(0, rt, q)
                if (rt, q) == (3, 3):
                    fetch_w(1, (0, 1))
            # conv1 (co-tiles 0,1 = all 256)
            for rt, q in sweep(1, 0, 1):
                conv_rt(1, rt, q)

    nc.finalize()
    return nc


_CACHED_NC = None


def _get_nc():
    global _CACHED_NC
    if _CACHED_NC is None:
        _CACHED_NC = build_nc()
    return _CACHED_NC


def _pack_wino(w, scale, n_cot):
    """[co, ci, 3, 3] -> [n_cot*4*128, 12*128] bf16: G-transformed along x,
    BN scale folded, blocks (q, ci_t)[ci_in, (p, dy), co_in]."""
    G = np.array([[1, 0, 0], [.5, .5, .5], [.5, -.5, .5], [0, 0, 1]],
                 np.float32)
    U = np.einsum('pk,oidk->pdoi', G, w.astype(np.float32))   # [4,3,co,ci]
    U = U * scale[None, None, :, None]
    U[2] = -U[2]   # stage-B: [ta1|tb1]=[m0|m1]+[m1|m2'], [ta|tb]-=[m2'|m3]
    A = U.transpose(3, 0, 1, 2)                               # [ci,p,dy,co]
    A = A.reshape(4, 128, 4, 3, n_cot, 128)
    A = A.transpose(4, 0, 1, 2, 3, 5)            # q,cit,ciin,p,dy,coin
    return np.ascontiguousarray(A.astype(BF)).reshape(n_cot * 4 * 128,
                                                      12 * 128)


def kernel(**inputs) -> np.ndarray:
    xt = np.asarray(inputs["xt"], np.float32)     # [8,512,32,32]
    xm = np.asarray(inputs["xm"], np.float32)     # [8,512,64,64]
    alpha = float(np.asarray(inputs["alpha"], np.float32).reshape(1)[0])
    w0 = np.asarray(inputs["w0"], np.float32)
    w1 = np.asarray(inputs["w1"], np.float32)
    g0 = np.asarray(inputs["g0"], np.float32)
    v0 = np.asarray(inputs["v0"], np.float32)
    g1 = np.asarray(inputs["g1"], np.float32)
    v1 = np.asarray(inputs["v1"], np.float32)

    s0 = g0 / np.sqrt(v0 + EPS)
    s1 = g1 / np.sqrt(v1 + EPS)
    w0p = _pack_wino(w0, s0, 4)
    w1p = _pack_wino(w1, s1, 2)

    kk = np.arange(0, 32, dtype=np.float32)
    wxeP = np.where(kk >= 1, -(kk / 63.0), 0.0)       # xhE[k]=xt[k]+wxeP[k]*d[k-1]
    wxoP = np.where(kk <= 30, (31 - kk) / 63.0, 0.0)  # xhO[k]=xt[k]+wxoP[k]*d[k]
    patx = np.concatenate([wxeP, wxoP]).astype(BF)
    # wyi: per j, [wye[j] x64 | wyo[j] x64]; wye[0]=0, wyo[31]=0 absorb edges
    wye = np.where(kk >= 1, -(kk / 63.0), 0.0)
    wyo = np.where(kk <= 30, (31 - kk) / 63.0, 0.0)
    paty = np.stack([np.repeat(wye, 64).reshape(32, 64),
                     np.repeat(wyo, 64).reshape(32, 64)],
                    axis=1).reshape(-1).astype(BF)    # [32*2*64]

    # alpha folded into xm; cols parity-blocked [E(32) | O(32)]
    xmeo = np.empty((N_CORES, 512, 64, 64), np.float32)
    xmeo[:, :, :, 0:32] = alpha * xm[:, :, :, 0::2]
    xmeo[:, :, :, 32:64] = alpha * xm[:, :, :, 1::2]
    xmeo = xmeo.astype(BF)
    xtb = xt.astype(BF)

    common = {"patx": patx, "paty": paty, "w0p": w0p, "w1p": w1p}
    for nm in ("g0", "b0", "m0", "v0"):
        common[nm] = np.asarray(inputs[nm], np.float32).reshape(512, 1)
    for nm in ("g1", "b1", "m1", "v1"):
        common[nm] = np.asarray(inputs[nm], np.float32).reshape(256, 1)

    in_maps = []
    for b in range(N_CORES):
        m = dict(common)
        m["xt"] = np.ascontiguousarray(xtb[b].reshape(512, 1024))
        m["xmeo"] = np.ascontiguousarray(xmeo[b].reshape(512, 4096))
        in_maps.append(m)

    nc = _get_nc()
    res = run_bass_kernel_spmd(nc, in_maps, core_ids=list(range(N_CORES)))
    out = np.stack([res.results[b]["out"].reshape(256, 64, 64)
                    for b in range(N_CORES)], axis=0)
    return out.astype(np.float32)


# revision 34
# speedup vs baseline: 1.0056x; 1.0017x over previous
"""Trainium2 Bass kernel for nn_Decoder_51539607552479.

DecoderModule.forward: bilinear-upsample xt (32->64, align_corners) ->
xfuse = xup + alpha*xm -> conv3x3(512->512)+BN+ReLU -> conv3x3(512->256)
+BN+ReLU.  Pure data parallel: batch dim (8) across the 8 NeuronCores,
weights replicated.

1D Winograd F(2,3) along x (direct 3-tap accumulation along y in PSUM)
with bf16 matmuls: 12 accumulating matmuls per output row-tile per
x-position instead of 36 direct taps -> 1.5x fewer PE rows; bf16
elementwise ops run in DVE 2x/4x perf modes (contiguous last dim).

dataflow per core (one image):
  DVE : bilinear upsample+fuse (bf16, parity-blocked cols) -> xloc tiles;
        inverse transform y0=m0+m1+m2, y1=m1-m2-m3 (+BN shift, ReLU)
        from Act-drained bf16 m -> eo tiles / output staging.
  Pool: input transforms V[p] = B^T d (4 col combos per row) from
        xloc/eo tiles into persistent pos-major V planes (no halo:
        x-transform is per-row; y-taps read adjacent plane rows).
  PE  : m_p += U[p,dy]^T V[p] (4 pos x 3 dy x 4 ci accumulating
        matmuls per row tile per co-tile), bf16.
  Act : PSUM->bf16 SBUF drain of all 4 positions in one op.
  host: Winograd weight transform (G w along x), BN scale folded into
        weights, alpha folded into xm, bf16 casts, parity split (free).
"""
import sys

if '/opt/trn_rl_repo' not in sys.path:
    sys.path.insert(0, '/opt/trn_rl_repo')

import numpy as np
import ml_dtypes
import concourse.bacc as bacc
import concourse.mybir as mybir
from concourse.ap import AP
from concourse.tile import TileContext
from concourse.bass_utils import run_bass_kernel_spmd

F32 = mybir.dt.float32
BF16 = mybir.dt.bfloat16
ALU = mybir.AluOpType
ACTF = mybir.ActivationFunctionType
EPS = 1e-5
BF = ml_dtypes.bfloat16

ROW_TILES = [(6 * i, 6) for i in range(10)] + [(60, 4)]
N_CORES = 8
UP_CHUNKS = [(0, 16), (16, 16), (32, 16), (48, 16)]
VP = 66 * 32               # one position's rows in a V plane
# xloc/eo blocked layout: cols 0..32 = even-x plane E[t] (x=2t, pad t=32),
# cols 33..65 = odd-x plane O[t] (x=2t-1, pad t=0)
EB = 0                     # E base col
OB = 33                    # O base col


def _v(ap2d, offset, rows, rowstep, cols):
    """[128, rows, cols] strided view of a [128, L] AP starting at offset."""
    sl = ap2d[:, offset: offset + rows * rowstep]
    return sl.rearrange("p (r c) -> p r c", c=rowstep)[:, :, 0:cols]


def build_nc():
    nc = bacc.Bacc(None, target_bir_lowering=True)

    xt_d = nc.dram_tensor("xt", [512, 1024], BF16, kind="ExternalInput")
    xm_d = nc.dram_tensor("xmeo", [512, 4096], BF16, kind="ExternalInput")
    patx_d = nc.dram_tensor("patx", [64], BF16, kind="ExternalInput")
    paty_d = nc.dram_tensor("paty", [64 * 64], BF16, kind="ExternalInput")
    w0_d = nc.dram_tensor("w0p", [16 * 128, 12 * 128], BF16,
                          kind="ExternalInput")
    w1_d = nc.dram_tensor("w1p", [8 * 128, 12 * 128], BF16,
                          kind="ExternalInput")
    bn_d = {}
    for nm in ("g0", "b0", "m0", "v0"):
        bn_d[nm] = nc.dram_tensor(nm, [512, 1], F32, kind="ExternalInput")
    for nm in ("g1", "b1", "m1", "v1"):
        bn_d[nm] = nc.dram_tensor(nm, [256, 1], F32, kind="ExternalInput")
    out_d = nc.dram_tensor("out", [256, 4096], F32, kind="ExternalOutput")

    with TileContext(nc) as tc:
        with tc.tile_pool(name="main", bufs=1) as P, \
             tc.tile_pool(name="wp", bufs=9) as WP, \
             tc.tile_pool(name="xtp", bufs=2) as XTP, \
             tc.tile_pool(name="xmp", bufs=2) as XMP, \
             tc.tile_pool(name="xhp", bufs=1) as XHP, \
             tc.tile_pool(name="xlp", bufs=2) as XLP, \
             tc.tile_pool(name="eop", bufs=4) as EOP, \
             tc.tile_pool(name="tmp", bufs=1) as TMP, \
             tc.tile_pool(name="mbp", bufs=6) as MBP, \
             tc.tile_pool(name="sbp", bufs=2) as SBP, \
             tc.tile_pool(name="outp", bufs=2) as OUTP, \
             tc.tile_pool(name="psum", bufs=4, space="PSUM") as PS:

            # ---------------- DRAM fetch helpers ----------------
            xt_tiles = {}
            xm_tiles = {}

            def xt_rows(c):
                r0, nr = UP_CHUNKS[c]
                j0, j1 = r0 // 2, (r0 + nr) // 2
                return max(j0 - 1, 0), min(j1 + 1, 32)

            def fetch_chunk(ct, c):
                jstart, jstop = xt_rows(c)
                ny = jstop - jstart
                t = XTP.tile([128, 10 * 32], BF16, tag="xt",
                             name=f"xt{ct}_{c}")
                nc.sync.dma_start(t[:, 0:ny * 32],
                                  xt_d[ct * 128:(ct + 1) * 128,
                                       jstart * 32: jstop * 32])
                xt_tiles[(ct, c)] = t
                r0, nr = UP_CHUNKS[c]
                tm = XMP.tile([128, 16 * 64 + 160], BF16, tag="xm",
                              name=f"xm{ct}_{c}")
                nc.sync.dma_start(tm[:, 0:nr * 64],
                                  xm_d[ct * 128:(ct + 1) * 128,
                                       r0 * 64:(r0 + nr) * 64])
                xm_tiles[(ct, c)] = tm

            patx = P.tile([128, 64], BF16, tag="patx")
            nc.sync.dma_start(patx[:], patx_d[:].partition_broadcast(128))
            paty = P.tile([128, 64 * 64], BF16, tag="paty")
            nc.sync.dma_start(paty[:], paty_d[:].partition_broadcast(128))

            def _cap(tile, off, dims):
                base = tile[:]
                pdim = list(list(base.ap)[0])
                return AP(base.tensor, base.offset + off,
                          [pdim] + [[s, n] for s, n in dims])
            for ct in range(4):
                fetch_chunk(ct, 0)

            # conv0 sweep-A weights (co-tiles 0,1)
            wt = {}

            def fetch_w(layer, qs):
                w_dram = w0_d if layer == 0 else w1_d
                for q in qs:
                    for ci in range(4):
                        t = WP.tile([128, 12 * 128], BF16, tag="w",
                                    name=f"w{layer}_{q}_{ci}")
                        row0 = (q * 4 + ci) * 128
                        nc.sync.dma_start(t[:], w_dram[row0:row0 + 128, :])
                        wt[(layer, q, ci)] = t

            fetch_w(0, (0, 1))

            bnp = {}
            for layer, n_cot in ((0, 4), (1, 2)):
                for q in range(n_cot):
                    sl = slice(q * 128, (q + 1) * 128)
                    for nm in ("g", "b", "m", "v"):
                        t = P.tile([128, 1], F32, tag=f"bn{nm}{layer}_{q}")
                        nc.sync.dma_start(t[:], bn_d[f"{nm}{layer}"][sl, :])
                        bnp[(layer, q, nm)] = t

            # ---------------- PE warmup ----------------
            wscr = P.tile([128, 640], BF16, tag="wscr")
            nc.gpsimd.memset(wscr[:], 0.0)
            pw = PS.tile([128, 1024], F32, tag="cpsum", name="pwarm")
            for wi in range(46):
                nc.tensor.matmul(pw[:, 0:512], wscr[:, 0:128],
                                 wscr[:, 128:640],
                                 start=True, stop=True, skip_group_check=True)

            # ---------------- persistent V planes ----------------
            # V plane: [128, 4 pos * 66 rows * 32], row pr = image row + 1
            V0 = [P.tile([128, 4 * VP], BF16, tag=f"v0_{i}", name=f"v0_{i}")
                  for i in range(4)]
            V1 = [P.tile([128, 4 * VP], BF16, tag=f"v1_{i}", name=f"v1_{i}")
                  for i in range(4)]
            for t in V0 + V1:
                # zero pad rows (pr 0 and 65) for all 4 positions.
                # True memset (not mul-by-0: SBUF garbage can be NaN).
                tv = t[:, 0:4 * VP].rearrange("p (g r) -> p g r", r=VP)
                nc.gpsimd.memset(tv[:, :, 0:32], 0.0)
                nc.gpsimd.memset(tv[:, :, 65 * 32:VP], 0.0)

            # ---------------- BN shift vectors ----------------
            shift = {}

            def emit_bn():
                for layer, n_cot in ((0, 4), (1, 2)):
                    for q in range(n_cot):
                        t = P.tile([128, 1], F32, tag=f"bnt{layer}_{q}")
                        sh = P.tile([128, 1], F32, tag=f"sh{layer}_{q}")
                        nc.vector.tensor_scalar_add(
                            t[:], bnp[(layer, q, "v")][:], EPS)
                        nc.scalar.activation(t[:], t[:], ACTF.Sqrt)
                        nc.vector.reciprocal(t[:], t[:])
                        nc.vector.tensor_mul(t[:], bnp[(layer, q, "g")][:],
                                             t[:])
                        nc.vector.tensor_mul(t[:], bnp[(layer, q, "m")][:],
                                             t[:])
                        nc.vector.tensor_sub(sh[:], bnp[(layer, q, "b")][:],
                                             t[:])
                        shift[(layer, q)] = sh

            # ---------------- x transform (B^T d), Pool ----------------
            def v_rows(src, splane, prow0, nrows, rstep, name,
                       eng=None):
                """src: blocked E|O tile view base AP [128, L]; writes
                V[p] rows prow0..prow0+nrows of plane splane."""
                eng = eng or nc.gpsimd
                E = _v(src, EB, nrows, rstep, 33)
                O = _v(src, OB, nrows, rstep, 33)
                vv = [_v(splane, p * VP + prow0 * 32, nrows, 32, 32)
                      for p in range(4)]
                eng.tensor_sub(vv[0], O[:, :, 0:32], O[:, :, 1:33])
                eng.tensor_add(vv[1], E[:, :, 0:32], O[:, :, 1:33])
                eng.tensor_sub(vv[2], O[:, :, 1:33], E[:, :, 0:32])
                eng.tensor_sub(vv[3], E[:, :, 0:32], E[:, :, 1:33])

            # ---------------- upsample + fuse (DVE, bf16) ----------------
            def upsample_chunk(ct, c, eng=None):
                if (ct, c) not in xt_tiles:
                    fetch_chunk(ct, c)
                r0, nrow = UP_CHUNKS[c]
                j0, j1 = r0 // 2, (r0 + nrow) // 2
                jstart, jstop = xt_rows(c)
                ny = jstop - jstart
                nD = ny - 1
                xt4 = xt_tiles[(ct, c)][:, 0:ny * 32] \
                    .rearrange("p (r c) -> p r c", c=32)
                xm_t = xm_tiles[(ct, c)]
                eng = eng or nc.vector
                # ---- x interp -> xh blocked [p, ny, 64] (E 0:32, O 32:64)
                # d layout [p, ny, 33]: col0 pad, cols1..32 = d[0..31]
                d = TMP.tile([128, 10 * 33], BF16, tag="d")
                d3 = d[:, 0:ny * 33].rearrange("p (r c) -> p r c", c=33)
                nc.gpsimd.memset(_cap(d, 0, [(33, ny), (32, 2)]), 0.0)
                eng.tensor_sub(d3[:, :, 1:32], xt4[:, :, 1:32],
                               xt4[:, :, 0:31])
                xh = XHP.tile([128, 10 * 64], BF16, tag="xh")
                xh4 = xh[:, 0:ny * 64].rearrange("p (r c) -> p r c", c=64)
                # merged E|O: xh[., s, k] = xt[., k] + wx[s, k]*d3[., s+k]
                xh_eo = _cap(xh, 0, [(64, ny), (32, 2), (1, 32)])
                eng.tensor_mul(xh_eo,
                               _cap(d, 0, [(33, ny), (1, 2), (1, 32)]),
                               _cap(patx, 0, [(0, ny), (32, 2), (1, 32)]))
                eng.tensor_add(xh_eo, xh_eo,
                               _cap(xt_tiles[(ct, c)],
                                    (jstart - jstart) * 32,
                                    [(32, ny), (0, 2), (1, 32)]))
                # ---- y interp + fuse -> xloc blocked [p, 16, 66]
                xl = XLP.tile([128, 16 * 66 + 176], BF16, tag="xl",
                              name=f"xl{ct}_{c}")
                nc.gpsimd.memset(_cap(xl, 32, [(66, 16), (1, 2)]), 0.0)
                # Dh rows at slot (j - (j0-1)); slot 0 / last may be dummy
                Dh = TMP.tile([128, 10 * 64], BF16, tag="dh")
                dh_off = 1 if c == 0 else 0
                if c == 0:
                    nc.gpsimd.memset(Dh[:, 0:64], 0.0)     # Dh[-1] (wye[0]=0)
                if c == 3:
                    nc.gpsimd.memset(Dh[:, 8 * 64:9 * 64], 0.0)  # Dh[31]
                eng.tensor_sub(Dh[:, dh_off * 64: (dh_off + nD) * 64],
                               xh[:, 64:(nD + 1) * 64], xh[:, 0:nD * 64])
                # unified even+odd rows: X[2j+k] = xh[j] + wyi[j,k]*Dh[j-1+k]
                #                                + xm[2j+k]
                ty = TMP.tile([128, 16 * 64], BF16, tag="ty")
                ty4 = _cap(ty, 0, [(128, 8), (64, 2), (1, 64)])
                eng.tensor_mul(ty4,
                               _cap(Dh, 0, [(64, 8), (64, 2), (1, 64)]),
                               _cap(paty, j0 * 128,
                                    [(128, 8), (64, 2), (1, 64)]))
                xo = (j0 - jstart) * 64
                eng.tensor_add(ty4, ty4,
                               _cap(xh, xo, [(64, 8), (0, 2), (1, 64)]))
                # dest: E cols at +0, O cols at +34, one 4D op incl +xm
                eng.tensor_add(_cap(xl, EB, [(66, 16), (34, 2), (1, 32)]),
                               _cap(ty, 0, [(64, 16), (32, 2), (1, 32)]),
                               _cap(xm_t, 0, [(64, 16), (32, 2), (1, 32)]))
                # ---- V0 rows for this chunk (DVE)
                v_rows(xl, V0[ct], r0 + 1, 16, 66, f"v0_{ct}_{c}",
                       eng=nc.vector)

            # ---------------- conv row-tile ----------------
            def conv_rt(layer, rt, q):
                r0, nr = ROW_TILES[rt]
                vsrc = V0 if layer == 0 else V1
                pt = PS.tile([128, 1024], F32, tag="cpsum",
                             name=f"ps{layer}_{rt}_{q}")
                for p in range(4):
                    ov = pt[:, p * 256: p * 256 + nr * 32] \
                        .rearrange("p (r c) -> p r c", c=32)
                    for dy in range(3):
                        for ci in range(4):
                            lhsT = wt[(layer, q, ci)][:, (p * 3 + dy) * 128:
                                                      (p * 3 + dy + 1) * 128]
                            rhs = _v(vsrc[ci], p * VP + (r0 + dy) * 32,
                                     nr, 32, 32)
                            nc.tensor.matmul(ov, lhsT, rhs,
                                             start=(dy == 0 and ci == 0),
                                             stop=(dy == 2 and ci == 3),
                                             skip_group_check=True)
                # drain 4 positions psum -> bf16 in one Act op
                mb = MBP.tile([128, 4 * 192], BF16, tag="mb",
                              name=f"mb{layer}_{rt}_{q}")
                n = nr * 32
                nc.scalar.activation(
                    mb[:, 0:4 * n].rearrange("p (g c) -> p g c", c=n),
                    _v(pt, 0, 4, 256, n), ACTF.Copy)
                # inverse transform + BN shift + ReLU on DVE
                eng = nc.vector
                # m layout [m0|m1|m2'|m3] with m2' = -m2 (host-negated):
                # [ta1|tb1] = [m0|m1]+[m1|m2'];  [ta|tb] = [ta1|tb1]-[m2'|m3]
                tab = SBP.tile([128, 384], BF16, tag="ta")
                eng.tensor_add(tab[:, 0:2 * n], mb[:, 0:2 * n],
                               mb[:, n:3 * n])
                eng.tensor_sub(tab[:, 0:2 * n], tab[:, 0:2 * n],
                               mb[:, 2 * n:4 * n])
                sh = shift[(layer, q)]
                tav = tab[:, 0:n].rearrange("p (r c) -> p r c", c=32)
                tbv = tab[:, n:2 * n].rearrange("p (r c) -> p r c", c=32)
                if layer == 0:
                    eo = EOP.tile([128, 6 * 66 + 80], BF16, tag="eo",
                                  name=f"eo{rt}_{q}")
                    nc.gpsimd.memset(_v(eo, 32, nr, 66, 2), 0.0)
                    eng.tensor_scalar(_v(eo, EB, nr, 66, 32), tav,
                                      sh[:, 0:1], 0.0, ALU.add, ALU.max)
                    eng.tensor_scalar(_v(eo, OB + 1, nr, 66, 32), tbv,
                                      sh[:, 0:1], 0.0, ALU.add, ALU.max)
                    # conv1 input transform rows for (rt, ci=q) on Pool
                    v_rows(eo, V1[q], r0 + 1, nr, 66, f"v1_{rt}_{q}")
                else:
                    ob = OUTP.tile([128, 384], F32, tag="ob",
                                   name=f"ob_{rt}_{q}")
                    ov4 = ob[:, 0:nr * 64].rearrange("p (r c t) -> p r c t",
                                                     c=32, t=2)
                    eng.tensor_scalar(ov4[:, :, :, 0:1].squeeze(), tav,
                                      sh[:, 0:1], 0.0, ALU.add, ALU.max)
                    eng.tensor_scalar(ov4[:, :, :, 1:2].squeeze(), tbv,
                                      sh[:, 0:1], 0.0, ALU.add, ALU.max)
                    nc.sync.dma_start(
                        out_d[q * 128:(q + 1) * 128,
                              r0 * 64:(r0 + nr) * 64],
                        ob[:, 0:nr * 64])

            # ---------------- emission ----------------
            def up_set(c):
                for ct in range(4):
                    upsample_chunk(ct, c)

            up_set(0)
            up_set(1)
            up_set(2)

            # sweep order: rt-major, but the last two row tiles complete one
            # co-tile at a time so its weight slots free early for the next
            # sweep's DMA.
            def sweep(layer, qa, qb):
                order = [(rt, q) for rt in range(9) for q in (qa, qb)]
                order += [(9, qa), (10, qa), (9, qb), (10, qb)]
                return order

            emit_bn()
            # conv0 sweep A (co-tiles 0,1)
            for rt, q in sweep(0, 0, 1):
                conv_rt(0, rt, q)
                if (rt, q) == (0, 1):
                    up_set(3)
                elif (rt, q) == (2, 1):
                    fetch_w(0, (2, 3))
            # conv0 sweep B (co-tiles 2,3)
            for rt, q in sweep(0, 2, 3):
                conv_rt(0, rt, q)
                if (rt, q) == (3, 3):
                    fetch_w(1, (0, 1))
            # conv1 (co-tiles 0,1 = all 256)
            for rt, q in sweep(1, 0, 1):
                conv_rt(1, rt, q)

    nc.finalize()
    return nc


_CACHED_NC = None


def _get_nc():
    global _CACHED_NC
    if _CACHED_NC is None:
        _CACHED_NC = build_nc()
    return _CACHED_NC


def _pack_wino(w, scale, n_cot):
    """[co, ci, 3, 3] -> [n_cot*4*128, 12*128] bf16: G-transformed along x,
    BN scale folded, blocks (q, ci_t)[ci_in, (p, dy), co_in]."""
    G = np.array([[1, 0, 0], [.5, .5, .5], [.5, -.5, .5], [0, 0, 1]],
                 np.float32)
    U = np.einsum('pk,oidk->pdoi', G, w.astype(np.float32))   # [4,3,co,ci]
    U = U * scale[None, None, :, None]
    U[2] = -U[2]   # stage-B: [ta1|tb1]=[m0|m1]+[m1|m2'], [ta|tb]-=[m2'|m3]
    A = U.transpose(3, 0, 1, 2)                               # [ci,p,dy,co]
    A = A.reshape(4, 128, 4, 3, n_cot, 128)
    A = A.transpose(4, 0, 1, 2, 3, 5)            # q,cit,ciin,p,dy,coin
    return np.ascontiguousarray(A.astype(BF)).reshape(n_cot * 4 * 128,
                                                      12 * 128)


def kernel(**inputs) -> np.ndarray:
    xt = np.asarray(inputs["xt"], np.float32)     # [8,512,32,32]
    xm = np.asarray(inputs["xm"], np.float32)     # [8,512,64,64]
    alpha = float(np.asarray(inputs["alpha"], np.float32).reshape(1)[0])
    w0 = np.asarray(inputs["w0"], np.float32)
    w1 = np.asarray(inputs["w1"], np.float32)
    g0 = np.asarray(inputs["g0"], np.float32)
    v0 = np.asarray(inputs["v0"], np.float32)
    g1 = np.asarray(inputs["g1"], np.float32)
    v1 = np.asarray(inputs["v1"], np.float32)

    s0 = g0 / np.sqrt(v0 + EPS)
    s1 = g1 / np.sqrt(v1 + EPS)
    w0p = _pack_wino(w0, s0, 4)
    w1p = _pack_wino(w1, s1, 2)

    kk = np.arange(0, 32, dtype=np.float32)
    wxeP = np.where(kk >= 1, -(kk / 63.0), 0.0)       # xhE[k]=xt[k]+wxeP[k]*d[k-1]
    wxoP = np.where(kk <= 30, (31 - kk) / 63.0, 0.0)  # xhO[k]=xt[k]+wxoP[k]*d[k]
    patx = np.concatenate([wxeP, wxoP]).astype(BF)
    # wyi: per j, [wye[j] x64 | wyo[j] x64]; wye[0]=0, wyo[31]=0 absorb edges
    wye = np.where(kk >= 1, -(kk / 63.0), 0.0)
    wyo = np.where(kk <= 30, (31 - kk) / 63.0, 0.0)
    paty = np.stack([np.repeat(wye, 64).reshape(32, 64),
                     np.repeat(wyo, 64).reshape(32, 64)],
                    axis=1).reshape(-1).astype(BF)    # [32*2*64]

    # alpha folded into xm; cols parity-blocked [E(32) | O(32)]
    xmeo = np.empty((N_CORES, 512, 64, 64), np.float32)
    xmeo[:, :, :, 0:32] = alpha * xm[:, :, :, 0::2]
    xmeo[:, :, :, 32:64] = alpha * xm[:, :, :, 1::2]
    xmeo = xmeo.astype(BF)
    xtb = xt.astype(BF)

    common = {"patx": patx, "paty": paty, "w0p": w0p, "w1p": w1p}
    for nm in ("g0", "b0", "m0", "v0"):
        common[nm] = np.asarray(inputs[nm], np.float32).reshape(512, 1)
    for nm in ("g1", "b1", "m1", "v1"):
        common[nm] = np.asarray(inputs[nm], np.float32).reshape(256, 1)

    in_maps = []
    for b in range(N_CORES):
        m = dict(common)
        m["xt"] = np.ascontiguousarray(xtb[b].reshape(512, 1024))
        m["xmeo"] = np.ascontiguousarray(xmeo[b].reshape(512, 4096))
        in_maps.append(m)

    nc = _get_nc()
    res = run_bass_kernel_spmd(nc, in_maps, core_ids=list(range(N_CORES)))
    out = np.stack([res.results[b]["out"].reshape(256, 64, 64)
                    for b in range(N_CORES)], axis=0)
    return out.astype(np.float32)
